# revision 8
# baseline (speedup 1.0000x reference)
"""Trainium2 Bass kernel for nn_Attention (dense transformer block-attention).

Reference semantics (faithful reshape WITHOUT head transpose):
  qkv = x @ w_qkv                    # [B, N, 3*1024]
  q = qkv[..., 0:1024].reshape(B, 16, 2048, 64)   # head h <- token rows [h*128,(h+1)*128)
  ...                                              # each row of 1024 = 16 sub-tokens of 64
  out[b, n, c] = O_head(n//128)[(n%128)*16 + c//64, c%64]

Sharding: 32 (b, head) pairs over 8 cores -> each core: 1 batch x 4 heads.
Pure data parallel, no collectives. Host preps xT (bf16) per core + full w (bf16).

Sub-token permutation: we process sub-tokens in order n2' = cb*128 + r
(instead of the reference's n2 = r*16 + cb). Softmax is permutation-
invariant over keys; queries are un-permuted via the output index mapping.
"""

import numpy as np
import ml_dtypes

B, N, D = 2, 2048, 1024
H_PER_CORE = 4          # head-blocks per core
ROWS = 128              # token rows per head-block
SUB = 2048              # sub-tokens per head (128 rows * 16 col-blocks)
DH = 64                 # head dim
CB = 16                 # col-blocks per row
SCALE = 0.125           # 64 ** -0.5
N_CORES = 8

_GRAPH = None  # (nc,) cached compiled graph


def build_graph():
    """Build + compile the 8-core SPMD Bass graph (same graph on every core)."""
    global _GRAPH
    if _GRAPH is not None:
        return _GRAPH

    import concourse.bass as bass
    import concourse.mybir as mybir
    import concourse.tile as tile
    from concourse import bacc
    from concourse.masks import make_identity
    from contextlib import ExitStack

    f32 = mybir.dt.float32
    bf16 = mybir.dt.bfloat16

    nc = bacc.Bacc("TRN2", target_bir_lowering=False, debug=False,
                   num_devices=N_CORES)

    xt_dram = nc.dram_tensor("xt", [D, H_PER_CORE * ROWS], bf16,
                             kind="ExternalInput")
    w_dram = nc.dram_tensor("w", [D, 3 * D], bf16, kind="ExternalInput")
    out_dram = nc.dram_tensor("out", [H_PER_CORE * ROWS, D], f32,
                              kind="ExternalOutput")

    KO = D // 128  # 8 k-tiles

    with tile.TileContext(nc) as tc, ExitStack() as ctx:
        const_pool = ctx.enter_context(tc.tile_pool(name="const", bufs=1))
        in_pool = ctx.enter_context(tc.tile_pool(name="inputs", bufs=1))
        qk_pool = ctx.enter_context(tc.tile_pool(name="qk", bufs=2))
        head_pool = ctx.enter_context(tc.tile_pool(name="head", bufs=1))
        pt_pool = ctx.enter_context(tc.tile_pool(name="pt", bufs=2))
        ot_pool = ctx.enter_context(tc.tile_pool(name="ot", bufs=1))
        small_pool = ctx.enter_context(tc.tile_pool(name="small", bufs=4))

        # ---- constants ----
        ident = const_pool.tile([128, 128], f32, tag="ident")
        make_identity(nc, ident[:])
        # warm up the exp activation table while projection runs
        warm = const_pool.tile([128, 1], f32, tag="warm")
        nc.vector.memset(warm[:], 0.0)
        nc.scalar.activation(warm[:], warm[:],
                             mybir.ActivationFunctionType.Exp)

        # ---- input DMA ----
        xt_sbuf = in_pool.tile([128, KO, H_PER_CORE * ROWS], bf16, tag="xt")
        w_sbuf = in_pool.tile([128, KO, 3 * D], bf16, tag="w")
        for ko in range(KO):
            nc.sync.dma_start(xt_sbuf[:, ko, :],
                              xt_dram.ap()[ko * 128:(ko + 1) * 128, :])
            nc.sync.dma_start(w_sbuf[:, ko, :],
                              w_dram.ap()[ko * 128:(ko + 1) * 128, :])

        # persistent per-head tiles.  qT/kT hold the head dim DUPLICATED on
        # both partition halves (partitions 0:64 and 64:128 carry the same
        # d-values), so the S matmul contracts K=128 and computes 2*(q.k);
        # the factor 2 is folded into the exp scale.
        qT = [head_pool.tile([128, SUB], bf16, tag=f"qT{t}", name=f"qT{t}")
              for t in range(H_PER_CORE)]
        kT = [head_pool.tile([128, SUB], bf16, tag=f"kT{t}", name=f"kT{t}")
              for t in range(H_PER_CORE)]
        v_ones = [head_pool.tile([128, CB, DH + 1], bf16, tag=f"vo{t}", name=f"vo{t}")
                  for t in range(H_PER_CORE)]
        for t in range(H_PER_CORE):
            nc.vector.memset(v_ones[t][:, :, DH], 1.0)

        # ---- phase 1+2: projection + q/k transposes + v relayout ----
        with tc.tile_pool(name="proj_psum", bufs=4, space="PSUM") as proj_psum:
            for t in range(H_PER_CORE):
                # q,k cast to bf16 with each 64-col block duplicated to 128
                qk2 = qk_pool.tile([128, 2 * CB, 128], bf16, tag="qk2")
                for ncx in range(6):
                    ps = proj_psum.tile([128, 512], f32, tag="proj")
                    for ko in range(KO):
                        nc.tensor.matmul(
                            ps[:],
                            xt_sbuf[:, ko, t * ROWS:(t + 1) * ROWS],
                            w_sbuf[:, ko, ncx * 512:(ncx + 1) * 512],
                            start=(ko == 0), stop=(ko == KO - 1))
                    if ncx < 4:  # q,k -> qk2 duplicated halves (bf16 cast)
                        src = ps[:].rearrange("p (a b) -> p a b", b=DH)
                        nc.vector.tensor_copy(
                            qk2[:, ncx * 8:(ncx + 1) * 8, 0:DH], src)
                        nc.vector.tensor_copy(
                            qk2[:, ncx * 8:(ncx + 1) * 8, DH:128], src)
                    else:        # v -> v_ones in [r, cb, d] layout
                        nc.vector.tensor_copy(
                            v_ones[t][:, (ncx - 4) * 8:(ncx - 3) * 8, 0:DH],
                            ps[:].rearrange("p (a b) -> p a b", b=DH))
                # q/k -> [d(dup), n2'] layout via DMA transpose
                for cb in range(CB):
                    nc.sync.dma_start_transpose(
                        qT[t][:, cb * 128:(cb + 1) * 128],
                        qk2[:, cb, :])
                    nc.sync.dma_start_transpose(
                        kT[t][:, cb * 128:(cb + 1) * 128],
                        qk2[:, CB + cb, :])

        # ---- phase 3: attention per head ----
        OT = [ot_pool.tile([128, SUB], f32, tag=f"OT{t}", name=f"OT{t}")
              for t in range(H_PER_CORE)]
        with tc.tile_pool(name="s_psum", bufs=1, space="PSUM") as s_psum, \
             tc.tile_pool(name="o_psum", bufs=1, space="PSUM") as o_psum:
            for t in range(H_PER_CORE):
                po = o_psum.tile([DH + 1, SUB], f32, tag="po")
                for j in range(CB):
                    ps = s_psum.tile([128, SUB], f32, tag="ps")
                    for ic in range(4):
                        nc.tensor.matmul(
                            ps[:, ic * 512:(ic + 1) * 512],
                            kT[t][:, j * 128:(j + 1) * 128],
                            qT[t][:, ic * 512:(ic + 1) * 512],
                            start=True, stop=True)
                    pt = pt_pool.tile([128, SUB], bf16, tag="pt")
                    # S psum holds 2*(q.k) due to duplicated heads -> scale/2
                    nc.scalar.activation(pt[:], ps[:],
                                         mybir.ActivationFunctionType.Exp,
                                         scale=SCALE / 2)
                    for ic in range(4):
                        nc.tensor.matmul(
                            po[:, ic * 512:(ic + 1) * 512],
                            v_ones[t][:, j, :],
                            pt[:, ic * 512:(ic + 1) * 512],
                            start=(j == 0), stop=(j == CB - 1))
                nc.vector.tensor_copy(OT[t][0:DH + 1, :], po[:])

        # ---- phase 4: transpose + normalize + write out ----
        with tc.tile_pool(name="tr_psum", bufs=2, space="PSUM") as tr_psum:
            for t in range(H_PER_CORE):
                for cb in range(CB):
                    ptr = tr_psum.tile([128, DH + 1], f32, tag="ptr")
                    nc.tensor.transpose(
                        ptr[:],
                        OT[t][0:DH + 1, cb * 128:(cb + 1) * 128],
                        ident[0:DH + 1, 0:DH + 1])
                    recip = small_pool.tile([128, 1], f32, tag="recip")
                    nc.vector.reciprocal(recip[:], ptr[:, DH:DH + 1])
                    outt = small_pool.tile([128, DH], f32, tag="outt")
                    nc.vector.tensor_scalar_mul(outt[:], ptr[:, 0:DH],
                                                recip[:])
                    nc.sync.dma_start(
                        out_dram.ap()[t * ROWS:(t + 1) * ROWS,
                                      cb * DH:(cb + 1) * DH],
                        outt[:])

    nc.compile()
    _GRAPH = nc
    return nc


def make_in_maps(x, w_qkv):
    w_bf = np.ascontiguousarray(w_qkv).astype(ml_dtypes.bfloat16)
    maps = []
    for c in range(N_CORES):
        b = c // 4
        r0 = (c % 4) * H_PER_CORE * ROWS
        xt = np.ascontiguousarray(
            x[b, r0:r0 + H_PER_CORE * ROWS, :].T).astype(ml_dtypes.bfloat16)
        maps.append({"xt": xt, "w": w_bf})
    return maps


def assemble_out(results):
    out = np.empty((B, N, D), dtype=np.float32)
    for c in range(N_CORES):
        b = c // 4
        r0 = (c % 4) * H_PER_CORE * ROWS
        out[b, r0:r0 + H_PER_CORE * ROWS, :] = results[c]["out"]
    return out


def kernel(x, w_qkv):
    from concourse import bass_utils
    nc = build_graph()
    res = bass_utils.run_bass_kernel_spmd(
        nc, make_in_maps(np.asarray(x), np.asarray(w_qkv)),
        list(range(N_CORES)))
    return assemble_out(res.results)


# revision 10
# speedup vs baseline: 1.1148x; 1.1148x over previous
"""Trainium2 Bass kernel for nn_Attention (dense transformer block-attention).

Reference semantics (faithful reshape WITHOUT head transpose):
  qkv = x @ w_qkv                    # [B, N, 3*1024]
  q = qkv[..., 0:1024].reshape(B, 16, 2048, 64)   # head h <- token rows [h*128,(h+1)*128)
  ...                                              # each row of 1024 = 16 sub-tokens of 64
  out[b, n, c] = O_head(n//128)[(n%128)*16 + c//64, c%64]

Sharding: 32 (b, head) pairs over 8 cores -> each core: 1 batch x 4 heads.
Pure data parallel, no collectives. Host preps xT (bf16) per core + full w (bf16).

Sub-token permutation: we process sub-tokens in order n2' = cb*128 + r
(instead of the reference's n2 = r*16 + cb). Softmax is permutation-
invariant over keys; queries are un-permuted via the output index mapping.
"""

import numpy as np
import ml_dtypes

B, N, D = 2, 2048, 1024
H_PER_CORE = 4          # head-blocks per core
ROWS = 128              # token rows per head-block
SUB = 2048              # sub-tokens per head (128 rows * 16 col-blocks)
DH = 64                 # head dim
CB = 16                 # col-blocks per row
SCALE = 0.125           # 64 ** -0.5
N_CORES = 8

_GRAPH = None  # (nc,) cached compiled graph


def build_graph():
    """Build + compile the 8-core SPMD Bass graph (same graph on every core)."""
    global _GRAPH
    if _GRAPH is not None:
        return _GRAPH

    import concourse.bass as bass
    import concourse.mybir as mybir
    import concourse.tile as tile
    from concourse import bacc
    from concourse.masks import make_identity
    from contextlib import ExitStack

    f32 = mybir.dt.float32
    bf16 = mybir.dt.bfloat16

    nc = bacc.Bacc("TRN2", target_bir_lowering=False, debug=False,
                   num_devices=N_CORES)

    xt_dram = nc.dram_tensor("xt", [D, H_PER_CORE * ROWS], bf16,
                             kind="ExternalInput")
    w_dram = nc.dram_tensor("w", [D, 3 * D], bf16, kind="ExternalInput")
    out_dram = nc.dram_tensor("out", [H_PER_CORE * ROWS, D], f32,
                              kind="ExternalOutput")

    KO = D // 128  # 8 k-tiles

    with tile.TileContext(nc) as tc, ExitStack() as ctx:
        const_pool = ctx.enter_context(tc.tile_pool(name="const", bufs=1))
        in_pool = ctx.enter_context(tc.tile_pool(name="inputs", bufs=1))
        qk_pool = ctx.enter_context(tc.tile_pool(name="qk", bufs=2))
        head_pool = ctx.enter_context(tc.tile_pool(name="head", bufs=1))
        pt_pool = ctx.enter_context(tc.tile_pool(name="pt", bufs=2))
        ot_pool = ctx.enter_context(tc.tile_pool(name="ot", bufs=1))
        small_pool = ctx.enter_context(tc.tile_pool(name="small", bufs=4))

        # ---- constants ----
        ident = const_pool.tile([128, 128], f32, tag="ident")
        make_identity(nc, ident[:])
        ident_bf = const_pool.tile([128, 128], bf16, tag="ident_bf")
        make_identity(nc, ident_bf[:])
        # warm up the exp activation table while projection runs
        warm = const_pool.tile([128, 1], f32, tag="warm")
        nc.vector.memset(warm[:], 0.0)
        nc.scalar.activation(warm[:], warm[:],
                             mybir.ActivationFunctionType.Exp)

        # ---- input DMA ----
        xt_sbuf = in_pool.tile([128, KO, H_PER_CORE * ROWS], bf16, tag="xt")
        w_sbuf = in_pool.tile([128, KO, 3 * D], bf16, tag="w")
        for ko in range(KO):
            nc.sync.dma_start(xt_sbuf[:, ko, :],
                              xt_dram.ap()[ko * 128:(ko + 1) * 128, :])
            nc.sync.dma_start(w_sbuf[:, ko, :],
                              w_dram.ap()[ko * 128:(ko + 1) * 128, :])

        # persistent per-head tiles.  qT/kT hold the head dim DUPLICATED on
        # both partition halves (partitions 0:64 and 64:128 carry the same
        # d-values), so the S matmul contracts K=128 and computes 2*(q.k);
        # the factor 2 is folded into the exp scale.
        qT = [head_pool.tile([128, SUB], bf16, tag=f"qT{t}", name=f"qT{t}")
              for t in range(H_PER_CORE)]
        kT = [head_pool.tile([128, SUB], bf16, tag=f"kT{t}", name=f"kT{t}")
              for t in range(H_PER_CORE)]
        v_ones = [head_pool.tile([128, CB, DH + 1], bf16, tag=f"vo{t}", name=f"vo{t}")
                  for t in range(H_PER_CORE)]
        for t in range(H_PER_CORE):
            nc.vector.memset(v_ones[t][:, :, DH], 1.0)

        # ---- phase 1+2: projection + q/k transposes + v relayout ----
        with tc.tile_pool(name="proj_psum", bufs=4, space="PSUM") as proj_psum, \
             tc.tile_pool(name="trq_psum", bufs=4, space="PSUM") as trq_psum:
            for t in range(H_PER_CORE):
                # q,k cast to bf16 with each 64-col block duplicated to 128
                qk2 = qk_pool.tile([128, 2 * CB, 128], bf16, tag="qk2")
                for ncx in range(6):
                    ps = proj_psum.tile([128, 512], f32, tag="proj")
                    for ko in range(KO):
                        nc.tensor.matmul(
                            ps[:],
                            xt_sbuf[:, ko, t * ROWS:(t + 1) * ROWS],
                            w_sbuf[:, ko, ncx * 512:(ncx + 1) * 512],
                            start=(ko == 0), stop=(ko == KO - 1))
                    if ncx < 4:  # q,k -> qk2 duplicated halves (bf16 cast)
                        src = ps[:].rearrange("p (a b) -> p a b", b=DH)
                        nc.vector.tensor_copy(
                            qk2[:, ncx * 8:(ncx + 1) * 8, 0:DH], src)
                        nc.vector.tensor_copy(
                            qk2[:, ncx * 8:(ncx + 1) * 8, DH:128], src)
                    else:        # v -> v_ones in [r, cb, d] layout (on ACT)
                        nc.scalar.copy(
                            v_ones[t][:, (ncx - 4) * 8:(ncx - 3) * 8, 0:DH],
                            ps[:].rearrange("p (a b) -> p a b", b=DH))
                # q/k -> [d(dup), n2'] layout via PE transpose
                for cb in range(2 * CB):
                    pst = trq_psum.tile([128, 128], bf16, tag="pst")
                    nc.tensor.transpose(pst[:], qk2[:, cb, :], ident_bf[:])
                    dst = qT[t] if cb < CB else kT[t]
                    nc.vector.tensor_copy(
                        dst[:, (cb % CB) * 128:((cb % CB) + 1) * 128], pst[:])

        # ---- phase 3: attention per head ----
        OT = [ot_pool.tile([128, SUB], f32, tag=f"OT{t}", name=f"OT{t}")
              for t in range(H_PER_CORE)]
        with tc.tile_pool(name="s_psum", bufs=1, space="PSUM") as s_psum, \
             tc.tile_pool(name="o_psum", bufs=1, space="PSUM") as o_psum:
            for t in range(H_PER_CORE):
                po = o_psum.tile([DH + 1, SUB], f32, tag="po")
                for j in range(CB):
                    ps = s_psum.tile([128, SUB], f32, tag="ps")
                    for ic in range(4):
                        nc.tensor.matmul(
                            ps[:, ic * 512:(ic + 1) * 512],
                            kT[t][:, j * 128:(j + 1) * 128],
                            qT[t][:, ic * 512:(ic + 1) * 512],
                            start=True, stop=True)
                    pt = pt_pool.tile([128, SUB], bf16, tag="pt")
                    # S psum holds 2*(q.k) due to duplicated heads -> scale/2
                    nc.scalar.activation(pt[:], ps[:],
                                         mybir.ActivationFunctionType.Exp,
                                         scale=SCALE / 2)
                    for ic in range(4):
                        nc.tensor.matmul(
                            po[:, ic * 512:(ic + 1) * 512],
                            v_ones[t][:, j, :],
                            pt[:, ic * 512:(ic + 1) * 512],
                            start=(j == 0), stop=(j == CB - 1))
                nc.vector.tensor_copy(OT[t][0:DH + 1, :], po[:])

        # ---- phase 4: transpose + normalize + write out ----
        with tc.tile_pool(name="tr_psum", bufs=2, space="PSUM") as tr_psum:
            for t in range(H_PER_CORE):
                for cb in range(CB):
                    ptr = tr_psum.tile([128, DH + 1], f32, tag="ptr")
                    nc.tensor.transpose(
                        ptr[:],
                        OT[t][0:DH + 1, cb * 128:(cb + 1) * 128],
                        ident[0:DH + 1, 0:DH + 1])
                    recip = small_pool.tile([128, 1], f32, tag="recip")
                    nc.vector.reciprocal(recip[:], ptr[:, DH:DH + 1])
                    outt = small_pool.tile([128, DH], f32, tag="outt")
                    nc.vector.tensor_scalar_mul(outt[:], ptr[:, 0:DH],
                                                recip[:])
                    nc.sync.dma_start(
                        out_dram.ap()[t * ROWS:(t + 1) * ROWS,
                                      cb * DH:(cb + 1) * DH],
                        outt[:])

    nc.compile()
    _GRAPH = nc
    return nc


def make_in_maps(x, w_qkv):
    w_bf = np.ascontiguousarray(w_qkv).astype(ml_dtypes.bfloat16)
    maps = []
    for c in range(N_CORES):
        b = c // 4
        r0 = (c % 4) * H_PER_CORE * ROWS
        xt = np.ascontiguousarray(
            x[b, r0:r0 + H_PER_CORE * ROWS, :].T).astype(ml_dtypes.bfloat16)
        maps.append({"xt": xt, "w": w_bf})
    return maps


def assemble_out(results):
    out = np.empty((B, N, D), dtype=np.float32)
    for c in range(N_CORES):
        b = c // 4
        r0 = (c % 4) * H_PER_CORE * ROWS
        out[b, r0:r0 + H_PER_CORE * ROWS, :] = results[c]["out"]
    return out


def kernel(x, w_qkv):
    from concourse import bass_utils
    nc = build_graph()
    res = bass_utils.run_bass_kernel_spmd(
        nc, make_in_maps(np.asarray(x), np.asarray(w_qkv)),
        list(range(N_CORES)))
    return assemble_out(res.results)


# revision 11
# speedup vs baseline: 1.3315x; 1.1944x over previous
"""Trainium2 Bass kernel for nn_Attention (dense transformer block-attention).

Reference semantics (faithful reshape WITHOUT head transpose):
  qkv = x @ w_qkv                    # [B, N, 3*1024]
  q = qkv[..., 0:1024].reshape(B, 16, 2048, 64)   # head h <- token rows [h*128,(h+1)*128)
  out[b, n, c] = O_head(n//128)[(n%128)*16 + c//64, c%64]

Sharding: 32 (b, head) pairs over 8 cores -> each core: 1 batch x 4 heads.
Pure data parallel, no collectives. Host preps xT (bf16) per core + full w (bf16).

Layout tricks:
- Sub-token permutation n2' = cb*128 + r (softmax is permutation-invariant
  over keys; queries un-permuted via the output index mapping).
- qT/kT hold the 64-wide head dim DUPLICATED on both partition halves, so
  S matmuls contract K=128 (computing 2*q.k; factor folded into exp scale)
  and the layout transposes are clean [128,128] PE transposes.
- PV: out^T = [v|ones].T @ exp(S^T): softmax denominators ride in row 64.
- One PSUM layout all kernel long: tag ps = 2x[128,1024] (4 banks) for
  projection accumulators / S ping-pong / tail transposes, tag po =
  1x[65,2048] (4 banks) for PV accumulators. No phase barriers.
"""

import numpy as np
import ml_dtypes

B, N, D = 2, 2048, 1024
H_PER_CORE = 4          # head-blocks per core
ROWS = 128              # token rows per head-block
SUB = 2048              # sub-tokens per head (128 rows * 16 col-blocks)
DH = 64                 # head dim
CB = 16                 # col-blocks per row
SCALE = 0.125           # 64 ** -0.5
N_CORES = 8

_GRAPH = None


def build_graph():
    global _GRAPH
    if _GRAPH is not None:
        return _GRAPH

    import concourse.mybir as mybir
    import concourse.tile as tile
    from concourse import bacc
    from concourse.masks import make_identity
    from contextlib import ExitStack

    f32 = mybir.dt.float32
    bf16 = mybir.dt.bfloat16
    EXP = mybir.ActivationFunctionType.Exp

    nc = bacc.Bacc("TRN2", target_bir_lowering=False, debug=False,
                   num_devices=N_CORES)

    xt_dram = nc.dram_tensor("xt", [D, H_PER_CORE * ROWS], bf16,
                             kind="ExternalInput")
    w_dram = nc.dram_tensor("w", [D, 3 * D], bf16, kind="ExternalInput")
    out_dram = nc.dram_tensor("out", [H_PER_CORE * ROWS, D], f32,
                              kind="ExternalOutput")

    KO = D // 128  # 8 k-tiles

    with tile.TileContext(nc) as tc, ExitStack() as ctx:
        const_pool = ctx.enter_context(tc.tile_pool(name="const", bufs=1))
        in_pool = ctx.enter_context(tc.tile_pool(name="inputs", bufs=1))
        qk_pool = ctx.enter_context(tc.tile_pool(name="qk", bufs=2))
        head_pool = ctx.enter_context(tc.tile_pool(name="head", bufs=1))
        pt_pool = ctx.enter_context(tc.tile_pool(name="pt", bufs=4))
        ot_pool = ctx.enter_context(tc.tile_pool(name="ot", bufs=1))
        small_pool = ctx.enter_context(tc.tile_pool(name="small", bufs=4))
        psum = ctx.enter_context(tc.tile_pool(name="psum", bufs=2,
                                              space="PSUM"))
        opsum = ctx.enter_context(tc.tile_pool(name="opsum", bufs=1,
                                               space="PSUM"))

        # ---- constants ----
        ident = const_pool.tile([128, 128], f32, tag="ident")
        make_identity(nc, ident[:])
        ident_bf = const_pool.tile([128, 128], bf16, tag="ident_bf")
        make_identity(nc, ident_bf[:])
        # warm up the exp table while the projection runs
        warm = const_pool.tile([128, 1], f32, tag="warm")
        nc.vector.memset(warm[:], 0.0)
        nc.scalar.activation(warm[:], warm[:], EXP)

        # ---- input DMA (per k-chunk so matmuls can start early) ----
        xt_sbuf = in_pool.tile([128, KO, H_PER_CORE * ROWS], bf16, tag="xt")
        w_sbuf = in_pool.tile([128, KO, 3 * D], bf16, tag="w")
        for ko in range(KO):
            nc.sync.dma_start(xt_sbuf[:, ko, :],
                              xt_dram.ap()[ko * 128:(ko + 1) * 128, :])
            nc.sync.dma_start(w_sbuf[:, ko, 0:2 * D],
                              w_dram.ap()[ko * 128:(ko + 1) * 128, 0:2 * D])
        for ko in range(KO):
            nc.sync.dma_start(w_sbuf[:, ko, 2 * D:3 * D],
                              w_dram.ap()[ko * 128:(ko + 1) * 128,
                                          2 * D:3 * D])

        # persistent per-head tiles (qT/kT carry duplicated d-halves)
        qT = [head_pool.tile([128, SUB], bf16, tag=f"qT{t}", name=f"qT{t}")
              for t in range(H_PER_CORE)]
        kT = [head_pool.tile([128, SUB], bf16, tag=f"kT{t}", name=f"kT{t}")
              for t in range(H_PER_CORE)]
        v_ones = [head_pool.tile([128, CB, DH + 1], bf16, tag=f"vo{t}",
                                 name=f"vo{t}")
                  for t in range(H_PER_CORE)]
        for t in range(H_PER_CORE):
            nc.vector.memset(v_ones[t][:, :, DH], 1.0)

        # ---- phase 1+2: projection + q/k transposes + v relayout ----
        for t in range(H_PER_CORE):
            qk2 = qk_pool.tile([128, 2 * CB, 128], bf16, tag="qk2")
            # q,k: cols 0:2048 -> two [128,1024] accumulators, ko-outer
            for half in range(2):
                ps = psum.tile([128, 1024], f32, tag="ps")
                for ko in range(KO):
                    for sub in range(2):
                        ncx = half * 2 + sub
                        nc.tensor.matmul(
                            ps[:, sub * 512:(sub + 1) * 512],
                            xt_sbuf[:, ko, t * ROWS:(t + 1) * ROWS],
                            w_sbuf[:, ko, ncx * 512:(ncx + 1) * 512],
                            start=(ko == 0), stop=(ko == KO - 1))
                for sub in range(2):
                    ncx = half * 2 + sub
                    src = ps[:, sub * 512:(sub + 1) * 512].rearrange(
                        "p (a b) -> p a b", b=DH)
                    nc.vector.tensor_copy(
                        qk2[:, ncx * 8:(ncx + 1) * 8, 0:DH], src)
                    nc.vector.tensor_copy(
                        qk2[:, ncx * 8:(ncx + 1) * 8, DH:128], src)
            # v: cols 2048:3072 -> one [128,1024] accumulator
            ps = psum.tile([128, 1024], f32, tag="ps")
            for ko in range(KO):
                for sub in range(2):
                    nc.tensor.matmul(
                        ps[:, sub * 512:(sub + 1) * 512],
                        xt_sbuf[:, ko, t * ROWS:(t + 1) * ROWS],
                        w_sbuf[:, ko, (4 + sub) * 512:(5 + sub) * 512],
                        start=(ko == 0), stop=(ko == KO - 1))
            nc.scalar.copy(
                v_ones[t][:, :, 0:DH],
                ps[:].rearrange("p (a b) -> p a b", b=DH))
            # q/k -> [d(dup), n2'] via PE transpose (borrow po slot when free)
            for cb in range(2 * CB):
                pst = psum.tile([128, 128], bf16, tag="ps")
                nc.tensor.transpose(pst[:], qk2[:, cb, :], ident_bf[:])
                dst = qT[t] if cb < CB else kT[t]
                nc.vector.tensor_copy(
                    dst[:, (cb % CB) * 128:((cb % CB) + 1) * 128], pst[:])

        # ---- phase 3: attention per head ----
        OT = [ot_pool.tile([128, SUB], f32, tag=f"OT{t}", name=f"OT{t}")
              for t in range(H_PER_CORE)]
        for t in range(H_PER_CORE):
            po = opsum.tile([DH + 1, SUB], f32, tag="po")
            for j in range(CB):
                for half in range(2):
                    ps = psum.tile([128, 1024], f32, tag="ps")
                    for sub in range(2):
                        ic = half * 2 + sub
                        nc.tensor.matmul(
                            ps[:, sub * 512:(sub + 1) * 512],
                            kT[t][:, j * 128:(j + 1) * 128],
                            qT[t][:, ic * 512:(ic + 1) * 512],
                            start=True, stop=True)
                    pt = pt_pool.tile([128, 1024], bf16, tag="pt")
                    # psum holds 2*(q.k) due to duplicated halves -> scale/2
                    nc.scalar.activation(pt[:], ps[:], EXP, scale=SCALE / 2)
                    for sub in range(2):
                        ic = half * 2 + sub
                        nc.tensor.matmul(
                            po[:, ic * 512:(ic + 1) * 512],
                            v_ones[t][:, j, :],
                            pt[:, sub * 512:(sub + 1) * 512],
                            start=(j == 0), stop=(j == CB - 1))
            nc.vector.tensor_copy(OT[t][0:DH + 1, :], po[:])

        # ---- phase 4: transpose + normalize + write out ----
        for t in range(H_PER_CORE):
            for cb in range(CB):
                ptr = psum.tile([128, DH + 1], f32, tag="ps")
                nc.tensor.transpose(
                    ptr[:],
                    OT[t][0:DH + 1, cb * 128:(cb + 1) * 128],
                    ident[0:DH + 1, 0:DH + 1])
                recip = small_pool.tile([128, 1], f32, tag="recip")
                nc.vector.reciprocal(recip[:], ptr[:, DH:DH + 1])
                outt = small_pool.tile([128, DH], f32, tag="outt")
                nc.vector.tensor_scalar_mul(outt[:], ptr[:, 0:DH], recip[:])
                nc.sync.dma_start(
                    out_dram.ap()[t * ROWS:(t + 1) * ROWS,
                                  cb * DH:(cb + 1) * DH],
                    outt[:])

    nc.compile()
    _GRAPH = nc
    return nc


def make_in_maps(x, w_qkv):
    w_bf = np.ascontiguousarray(w_qkv).astype(ml_dtypes.bfloat16)
    maps = []
    for c in range(N_CORES):
        b = c // 4
        r0 = (c % 4) * H_PER_CORE * ROWS
        xt = np.ascontiguousarray(
            x[b, r0:r0 + H_PER_CORE * ROWS, :].T).astype(ml_dtypes.bfloat16)
        maps.append({"xt": xt, "w": w_bf})
    return maps


def assemble_out(results):
    out = np.empty((B, N, D), dtype=np.float32)
    for c in range(N_CORES):
        b = c // 4
        r0 = (c % 4) * H_PER_CORE * ROWS
        out[b, r0:r0 + H_PER_CORE * ROWS, :] = results[c]["out"]
    return out


def kernel(x, w_qkv):
    from concourse import bass_utils
    nc = build_graph()
    res = bass_utils.run_bass_kernel_spmd(
        nc, make_in_maps(np.asarray(x), np.asarray(w_qkv)),
        list(range(N_CORES)))
    return assemble_out(res.results)


# revision 12
# speedup vs baseline: 1.5934x; 1.1967x over previous
"""Trainium2 Bass kernel for nn_Attention (dense transformer block-attention).

Reference semantics (faithful reshape WITHOUT head transpose):
  qkv = x @ w_qkv                    # [B, N, 3*1024]
  q = qkv[..., 0:1024].reshape(B, 16, 2048, 64)   # head h <- token rows [h*128,(h+1)*128)
  out[b, n, c] = O_head(n//128)[(n%128)*16 + c//64, c%64]

Sharding: 32 (b, head) pairs over 8 cores -> each core: 1 batch x 4 heads.
Pure data parallel, no collectives. Host preps xT (bf16) per core + full w (bf16).

Layout tricks:
- Sub-token permutation n2' = cb*128 + r (softmax is permutation-invariant
  over keys; queries un-permuted via the output index mapping).
- qT/kT hold the 64-wide head dim DUPLICATED on both partition halves, so
  S matmuls contract K=128 (computing 2*q.k; factor folded into exp scale)
  and the layout transposes are clean [128,128] PE transposes.
- PV: out^T = [v|ones].T @ exp(S^T): softmax denominators ride in row 64.
- One PSUM layout all kernel long: tag ps = 2x[128,1024] (4 banks) used by
  projection accumulators / S ping-pong / tail transposes, tag po =
  1x[65,2048] (4 banks) for PV accumulators. No phase barriers.
"""

import numpy as np
import ml_dtypes

B, N, D = 2, 2048, 1024
H_PER_CORE = 4          # head-blocks per core
ROWS = 128              # token rows per head-block
SUB = 2048              # sub-tokens per head (128 rows * 16 col-blocks)
DH = 64                 # head dim
CB = 16                 # col-blocks per row
SCALE = 0.125           # 64 ** -0.5
N_CORES = 8

_GRAPH = None


def build_graph():
    global _GRAPH
    if _GRAPH is not None:
        return _GRAPH

    import concourse.mybir as mybir
    import concourse.tile as tile
    from concourse import bacc
    from concourse.masks import make_identity
    from contextlib import ExitStack

    f32 = mybir.dt.float32
    bf16 = mybir.dt.bfloat16
    EXP = mybir.ActivationFunctionType.Exp

    nc = bacc.Bacc("TRN2", target_bir_lowering=False, debug=False,
                   num_devices=N_CORES)

    xt_dram = nc.dram_tensor("xt", [D, H_PER_CORE * ROWS], bf16,
                             kind="ExternalInput")
    w_dram = nc.dram_tensor("w", [D, 3 * D], bf16, kind="ExternalInput")
    out_dram = nc.dram_tensor("out", [H_PER_CORE * ROWS, D], f32,
                              kind="ExternalOutput")

    KO = D // 128  # 8 k-tiles

    with tile.TileContext(nc) as tc, ExitStack() as ctx:
        const_pool = ctx.enter_context(tc.tile_pool(name="const", bufs=1))
        in_pool = ctx.enter_context(tc.tile_pool(name="inputs", bufs=1))
        qk_pool = ctx.enter_context(tc.tile_pool(name="qk", bufs=4))
        head_pool = ctx.enter_context(tc.tile_pool(name="head", bufs=1))
        pt_pool = ctx.enter_context(tc.tile_pool(name="pt", bufs=4))
        ot_pool = ctx.enter_context(tc.tile_pool(name="ot", bufs=2))
        small_pool = ctx.enter_context(tc.tile_pool(name="small", bufs=4))
        psum = ctx.enter_context(tc.tile_pool(name="psum", bufs=2,
                                              space="PSUM"))
        opsum = ctx.enter_context(tc.tile_pool(name="opsum", bufs=1,
                                               space="PSUM"))

        # ---- constants ----
        ident = const_pool.tile([128, 128], f32, tag="ident")
        make_identity(nc, ident[:])
        ident_bf = const_pool.tile([128, 128], bf16, tag="ident_bf")
        make_identity(nc, ident_bf[:])
        # warm up the exp table while the projection runs
        warm = const_pool.tile([128, 1], f32, tag="warm")
        nc.vector.memset(warm[:], 0.0)
        nc.scalar.activation(warm[:], warm[:], EXP)

        # ---- input DMA: xt first, then w in consumption order ----
        xt_sbuf = in_pool.tile([128, KO, H_PER_CORE * ROWS], bf16, tag="xt")
        w_sbuf = in_pool.tile([128, KO, 3 * D], bf16, tag="w")
        for ko in range(KO):
            nc.sync.dma_start(xt_sbuf[:, ko, :],
                              xt_dram.ap()[ko * 128:(ko + 1) * 128, :])
        for half in range(3):
            for ko in range(KO):
                nc.sync.dma_start(
                    w_sbuf[:, ko, half * 1024:(half + 1) * 1024],
                    w_dram.ap()[ko * 128:(ko + 1) * 128,
                                half * 1024:(half + 1) * 1024])

        # persistent per-head tiles (qT/kT carry duplicated d-halves)
        qT = [head_pool.tile([128, SUB], bf16, tag=f"qT{t}", name=f"qT{t}")
              for t in range(H_PER_CORE)]
        kT = [head_pool.tile([128, SUB], bf16, tag=f"kT{t}", name=f"kT{t}")
              for t in range(H_PER_CORE)]
        v_ones = [head_pool.tile([128, CB, DH + 1], bf16, tag=f"vo{t}",
                                 name=f"vo{t}")
                  for t in range(H_PER_CORE)]
        for t in range(H_PER_CORE):
            nc.vector.memset(v_ones[t][:, :, DH], 1.0)

        # ---- phase 1: projection for all blocks ----
        qk2s = []
        for t in range(H_PER_CORE):
            qk2 = qk_pool.tile([128, 2 * CB, 128], bf16, tag="qk2",
                               name=f"qk2_{t}")
            qk2s.append(qk2)
            # q,k: cols 0:2048 -> two [128,1024] accumulators, ko-outer
            for half in range(2):
                ps = psum.tile([128, 1024], f32, tag="ps")
                for ko in range(KO):
                    for sub in range(2):
                        ncx = half * 2 + sub
                        nc.tensor.matmul(
                            ps[:, sub * 512:(sub + 1) * 512],
                            xt_sbuf[:, ko, t * ROWS:(t + 1) * ROWS],
                            w_sbuf[:, ko, ncx * 512:(ncx + 1) * 512],
                            start=(ko == 0), stop=(ko == KO - 1))
                for sub in range(2):
                    ncx = half * 2 + sub
                    src = ps[:, sub * 512:(sub + 1) * 512].rearrange(
                        "p (a b) -> p a b", b=DH)
                    nc.vector.tensor_copy(
                        qk2[:, ncx * 8:(ncx + 1) * 8, 0:DH], src)
                    nc.vector.tensor_copy(
                        qk2[:, ncx * 8:(ncx + 1) * 8, DH:128], src)
            # v: cols 2048:3072 -> one [128,1024] accumulator
            ps = psum.tile([128, 1024], f32, tag="ps")
            for ko in range(KO):
                for sub in range(2):
                    nc.tensor.matmul(
                        ps[:, sub * 512:(sub + 1) * 512],
                        xt_sbuf[:, ko, t * ROWS:(t + 1) * ROWS],
                        w_sbuf[:, ko, (4 + sub) * 512:(5 + sub) * 512],
                        start=(ko == 0), stop=(ko == KO - 1))
            nc.scalar.copy(
                v_ones[t][:, :, 0:DH],
                ps[:].rearrange("p (a b) -> p a b", b=DH))

        # ---- per head: transposes -> attention -> tail ----
        for t in range(H_PER_CORE):
            # q/k -> [d(dup), n2'] via PE transpose
            for cb in range(2 * CB):
                pst = psum.tile([128, 128], bf16, tag="ps")
                nc.tensor.transpose(pst[:], qk2s[t][:, cb, :], ident_bf[:])
                dst = qT[t] if cb < CB else kT[t]
                nc.vector.tensor_copy(
                    dst[:, (cb % CB) * 128:((cb % CB) + 1) * 128], pst[:])

            po = opsum.tile([DH + 1, SUB], f32, tag="po")
            for j in range(CB):
                for half in range(2):
                    ps = psum.tile([128, 1024], f32, tag="ps")
                    for sub in range(2):
                        ic = half * 2 + sub
                        nc.tensor.matmul(
                            ps[:, sub * 512:(sub + 1) * 512],
                            kT[t][:, j * 128:(j + 1) * 128],
                            qT[t][:, ic * 512:(ic + 1) * 512],
                            start=True, stop=True)
                    pt = pt_pool.tile([128, 1024], bf16, tag="pt")
                    # psum holds 2*(q.k) due to duplicated halves -> scale/2
                    nc.scalar.activation(pt[:], ps[:], EXP, scale=SCALE / 2)
                    for sub in range(2):
                        ic = half * 2 + sub
                        nc.tensor.matmul(
                            po[:, ic * 512:(ic + 1) * 512],
                            v_ones[t][:, j, :],
                            pt[:, sub * 512:(sub + 1) * 512],
                            start=(j == 0), stop=(j == CB - 1))
            OTt = ot_pool.tile([128, SUB], f32, tag="OT", name=f"OT{t}")
            nc.vector.tensor_copy(OTt[0:DH + 1, :], po[:])

            # tail: transpose + normalize + write out (overlaps next head)
            for cb in range(CB):
                ptr = psum.tile([128, DH + 1], f32, tag="ps")
                nc.tensor.transpose(
                    ptr[:],
                    OTt[0:DH + 1, cb * 128:(cb + 1) * 128],
                    ident[0:DH + 1, 0:DH + 1])
                recip = small_pool.tile([128, 1], f32, tag="recip")
                nc.vector.reciprocal(recip[:], ptr[:, DH:DH + 1])
                outt = small_pool.tile([128, DH], f32, tag="outt")
                nc.vector.tensor_scalar_mul(outt[:], ptr[:, 0:DH], recip[:])
                nc.sync.dma_start(
                    out_dram.ap()[t * ROWS:(t + 1) * ROWS,
                                  cb * DH:(cb + 1) * DH],
                    outt[:])

    nc.compile()
    _GRAPH = nc
    return nc


def make_in_maps(x, w_qkv):
    w_bf = np.ascontiguousarray(w_qkv).astype(ml_dtypes.bfloat16)
    maps = []
    for c in range(N_CORES):
        b = c // 4
        r0 = (c % 4) * H_PER_CORE * ROWS
        xt = np.ascontiguousarray(
            x[b, r0:r0 + H_PER_CORE * ROWS, :].T).astype(ml_dtypes.bfloat16)
        maps.append({"xt": xt, "w": w_bf})
    return maps


def assemble_out(results):
    out = np.empty((B, N, D), dtype=np.float32)
    for c in range(N_CORES):
        b = c // 4
        r0 = (c % 4) * H_PER_CORE * ROWS
        out[b, r0:r0 + H_PER_CORE * ROWS, :] = results[c]["out"]
    return out


def kernel(x, w_qkv):
    from concourse import bass_utils
    nc = build_graph()
    res = bass_utils.run_bass_kernel_spmd(
        nc, make_in_maps(np.asarray(x), np.asarray(w_qkv)),
        list(range(N_CORES)))
    return assemble_out(res.results)
